# revision 30
# baseline (speedup 1.0000x reference)
"""MoE FFN (8 experts, top-2, SwiGLU) Trainium2 kernel — bf16 half-expert.

Sharding: each expert's hidden dim H=2048 is split into two halves; the
16 (expert, H-half) jobs are packed 2-per-core. Slot 0 holds halves of
the 4 heaviest-loaded experts (per the host router) with token capacity
1092, slot 1 the 4 lightest with capacity 1024 — vs 1152 for plain
expert-parallel SPMD, a ~8% cut in padded matmul rows. The two halves
of an expert produce partial y sums the host adds during scatter.

The router (top-2 + combine weights) runs on host, exactly replicating
the reference. FFN matmuls are bf16 (measured PE rate: 1 cycle per
moving row, weight loads fully hidden), end-to-end rel err ~4.5e-3.
Structure per h-tile: a g-pass then v-pass over 8 K-tiles x 2-3 token
chunks sharing PSUM tags generationally, epilogue computes
silu(g)*v -> bf16 hT tiles consumed by phase B; y is stored bf16 and
scaled/accumulated on host.

Self-contained: shapes/sharding hardcoded for
x[2,2048,1024], 8 experts, d_expert=2048, top-2.
"""

import math
from contextlib import ExitStack

import ml_dtypes
import numpy as np

import concourse.mybir as mybir
import concourse.tile as tile
from concourse import bacc
from concourse.bass_utils import run_bass_kernel_spmd

# ---- problem constants --------------------------------------------------
B, T, D = 2, 2048, 1024
N_TOK = B * T          # 4096 tokens
E = 8                  # experts
H = 2048               # expert hidden dim
HH = H // 2            # per-slot hidden half
TOP_K = 2
P = 128

CAP0 = 1092            # slot-0 token capacity (4 heaviest experts)
CAP1 = 1024            # slot-1 token capacity (4 lightest experts)
ND = D // P            # 8  d-tiles
NHH = HH // P          # 8  h-tiles per slot
NT0 = (CAP0 + P - 1) // P   # 9 (last tile 68 tokens)
NT1 = CAP1 // P             # 8

BFD = mybir.dt.bfloat16
FP = mybir.dt.float32
AF = mybir.ActivationFunctionType
OP = mybir.AluOpType
BF = ml_dtypes.bfloat16

CH0 = [(0, 512), (512, 512), (1024, CAP0 - 1024)]
CH1 = [(0, 512), (512, 512)]

def _emit(nc, tc, ctx, tens):
    const = ctx.enter_context(tc.tile_pool(name="const", bufs=1))
    wsb = ctx.enter_context(tc.tile_pool(name="wsb", bufs=1))
    htp = ctx.enter_context(tc.tile_pool(name="htp", bufs=1))
    act = ctx.enter_context(tc.tile_pool(name="act", bufs=3))
    yst = ctx.enter_context(tc.tile_pool(name="yst", bufs=3))

    caps = (CAP0, CAP1)
    chunks = (CH0, CH1)
    nts = (NT0, NT1)
    x_sb, wg_sb, wv_sb, wo_sb = [], [], [], []
    x_ap, wg_ap, wv_ap, wo_ap = [], [], [], []
    for s in range(2):
        x_sb.append(const.tile([P, ND, caps[s]], BFD, name=f"x{s}"))
        x_ap.append(tens[f"x{s}"].ap().rearrange("(j p) c -> p j c", p=P))
        wg_sb.append(wsb.tile([P, ND, HH], BFD, name=f"wg{s}"))
        wg_ap.append(tens[f"wg{s}"].ap().rearrange("(j p) h -> p j h", p=P))
        wv_sb.append(wsb.tile([P, ND, HH], BFD, name=f"wv{s}"))
        wv_ap.append(tens[f"wv{s}"].ap().rearrange("(j p) h -> p j h", p=P))
        wo_sb.append(wsb.tile([P, NHH, D], BFD, name=f"wo{s}"))
        wo_ap.append(tens[f"wo{s}"].ap().rearrange("(j p) d -> p j d", p=P))

    # head-latency-ordered loads: sync carries the critical path in
    # first-use order (small first chunks -> first matmul asap); the
    # scalar queue configures the v-pass head load in parallel. Bulk
    # loads follow on sync so they cannot steal HBM bandwidth from the
    # head.
    nc.sync.dma_start(out=x_sb[0][:, 0:1, 0:512], in_=x_ap[0][:, 0:1, 0:512])
    nc.sync.dma_start(out=wg_sb[0][:, :, 0:P], in_=wg_ap[0][:, :, 0:P])
    nc.sync.dma_start(out=x_sb[0][:, 0:1, 512:CAP0], in_=x_ap[0][:, 0:1, 512:CAP0])
    nc.sync.dma_start(out=x_sb[0][:, 1:2], in_=x_ap[0][:, 1:2])
    nc.sync.dma_start(out=x_sb[0][:, 2:4], in_=x_ap[0][:, 2:4])
    nc.sync.dma_start(out=x_sb[0][:, 4:6], in_=x_ap[0][:, 4:6])
    nc.sync.dma_start(out=x_sb[0][:, 6:8], in_=x_ap[0][:, 6:8])
    nc.scalar.dma_start(out=wv_sb[0][:, :, 0:P], in_=wv_ap[0][:, :, 0:P])
    HQ = (HH - P) // 2 + P  # 576: split the bulk wg/wv columns in two
    nc.sync.dma_start(out=wg_sb[0][:, :, P:HQ], in_=wg_ap[0][:, :, P:HQ])
    nc.sync.dma_start(out=wv_sb[0][:, :, P:HQ], in_=wv_ap[0][:, :, P:HQ])
    nc.sync.dma_start(out=wg_sb[0][:, :, HQ:HH], in_=wg_ap[0][:, :, HQ:HH])
    nc.sync.dma_start(out=wv_sb[0][:, :, HQ:HH], in_=wv_ap[0][:, :, HQ:HH])
    nc.sync.dma_start(out=x_sb[1][:, 0:4], in_=x_ap[1][:, 0:4])
    nc.sync.dma_start(out=x_sb[1][:, 4:8], in_=x_ap[1][:, 4:8])
    nc.sync.dma_start(out=wg_sb[1][:], in_=wg_ap[1])
    nc.sync.dma_start(out=wv_sb[1][:], in_=wv_ap[1])
    nc.sync.dma_start(out=wo_sb[0][:], in_=wo_ap[0])
    nc.sync.dma_start(out=wo_sb[1][:], in_=wo_ap[1])

    ht = [[htp.tile([P, caps[s]], BFD, name=f"ht{s}_{k}") for k in range(NHH)]
          for s in range(2)]

    # PE p-state warm-up on a zeroed tile: no DMA dependency, so the
    # ramp starts as soon as the vector engine can memset (~1us in).
    warmz = const.tile([P, 2 * P], BFD)
    nc.vector.memset(warmz[:], 0.0)
    with ExitStack() as wctx:
        ps_w = wctx.enter_context(tc.tile_pool(name="psw", bufs=1, space="PSUM"))
        warm = ps_w.tile([P, 2 * P], FP, name="warm", tag="warm")
        for _ in range(24):
            nc.tensor.matmul(warm[:], lhsT=warmz[:, 0:P], rhs=warmz[:],
                             start=True, stop=True)

    # ---- phase A: hT[h, tok] = silu(x@wg)^T * (x@wv)^T ------------------
    # g-pass then v-pass share psum tags (generational cycling): sigmoid
    # and t1 run mid-tile on the g results, freeing banks early.
    with ExitStack() as actx:
        ps_a = actx.enter_context(tc.tile_pool(name="psa", bufs=3, space="PSUM"))
        for s in range(2):
            for hk in range(NHH):
                hs = slice(hk * P, (hk + 1) * P)
                pgs = [ps_a.tile([P, cw], FP, name=f"pg{ci}", tag=f"p{ci}",
                                 bufs=(2 if ci == 2 else 3))
                       for ci, (_, cw) in enumerate(chunks[s])]
                pvs = [ps_a.tile([P, cw], FP, name=f"pv{ci}", tag=f"p{ci}",
                                 bufs=(2 if ci == 2 else 3))
                       for ci, (_, cw) in enumerate(chunks[s])]
                if s == 0 and hk == 0:
                    # interleave g/v per j-tile: halves the x consumption
                    # rate so the first tile tracks the incoming x DMAs
                    for j in range(ND):
                        for w_s, pss in ((wg_sb, pgs), (wv_sb, pvs)):
                            lhsT = w_s[s][:, j, hs]
                            for ci, (cs, cw) in enumerate(chunks[s]):
                                nc.tensor.matmul(
                                    pss[ci][:], lhsT=lhsT,
                                    rhs=x_sb[s][:, j, cs:cs + cw],
                                    start=(j == 0), stop=(j == ND - 1))
                else:
                    for w_s, pss in ((wg_sb, pgs), (wv_sb, pvs)):
                        for j in range(ND):
                            lhsT = w_s[s][:, j, hs]
                            for ci, (cs, cw) in enumerate(chunks[s]):
                                nc.tensor.matmul(
                                    pss[ci][:], lhsT=lhsT,
                                    rhs=x_sb[s][:, j, cs:cs + cw],
                                    start=(j == 0), stop=(j == ND - 1))
                for ci, (cs, cw) in enumerate(chunks[s]):
                    sg = act.tile([P, 512], FP, tag="sg")
                    nc.scalar.activation(sg[:, :cw], pgs[ci][:], AF.Sigmoid)
                    t1 = act.tile([P, 512], FP, tag="t1")
                    nc.vector.tensor_tensor(t1[:, :cw], pgs[ci][:],
                                            sg[:, :cw], op=OP.mult)
                    nc.vector.tensor_tensor(ht[s][hk][:, cs:cs + cw],
                                            t1[:, :cw], pvs[ci][:],
                                            op=OP.mult)

    # ---- phase B: yT[d, tok] = wo^T @ hT --------------------------------
    # d-major output: wo tiles are the stationary operand swept over the
    # exact token count (no padded-token rows for the partial tile); the
    # per-token combine weight is applied by the host during scatter.
    with ExitStack() as bctx:
        ps_y = bctx.enter_context(tc.tile_pool(name="psy", bufs=3, space="PSUM"))
        for s in range(2):
            y_ap = tens[f"y{s}"].ap()
            for dt in range(ND):
                dsl = slice(dt * P, (dt + 1) * P)
                pys = [ps_y.tile([P, cw], FP, name=f"py{ci}", tag=f"py{ci}",
                                 bufs=(2 if ci == 2 else 3))
                       for ci, (_, cw) in enumerate(chunks[s])]
                for hj in range(NHH):
                    lhsT = wo_sb[s][:, hj, dsl]
                    for ci, (cs, cw) in enumerate(chunks[s]):
                        nc.tensor.matmul(
                            pys[ci][:], lhsT=lhsT,
                            rhs=ht[s][hj][:, cs:cs + cw],
                            start=(hj == 0), stop=(hj == NHH - 1))
                ysb = yst.tile([P, caps[s]], BFD, tag="y", name="ysb")
                for ci, (cs, cw) in enumerate(chunks[s]):
                    nc.scalar.activation(ysb[:, cs:cs + cw], pys[ci][:],
                                         AF.Copy)
                # alternate store queues: halves the serialized dma_start
                # sequencing in the drain past the last matmul
                eng = nc.gpsimd if (dt % 2 == 0) else nc.scalar
                eng.dma_start(out=y_ap[dsl, :], in_=ysb[:])


def _dedup_ldweights(nc):
    """Drop InstLdweights that reload the exact weights already resident
    in the PE array (identical AP/mode, no intervening clobber, no sync)."""
    removed = 0
    for blk in nc.main_func.blocks:
        last_key = None
        new = []
        for inst in blk.instructions:
            if isinstance(inst, mybir.InstLdweights):
                si = inst.sync_info
                clean = si is None or (not si.on_wait and not si.on_update)
                key = (
                    repr(inst.ins[0]), str(inst.perf_mode),
                    str(inst.is_transpose), str(inst.tile_position),
                    str(inst.tile_size),
                )
                if clean and key == last_key:
                    removed += 1
                    continue
                last_key = key
            elif isinstance(inst, mybir.InstMatmult):
                if inst.ldweights is None or inst.is_transpose:
                    last_key = None
            new.append(inst)
        blk.instructions[:] = new
    return removed


def _build():
    nc = bacc.Bacc("TRN2", target_bir_lowering=False, debug=False)
    tens = {
        "x0": nc.dram_tensor("x0", [D, CAP0], BFD, kind="ExternalInput"),
        "x1": nc.dram_tensor("x1", [D, CAP1], BFD, kind="ExternalInput"),
        "wg0": nc.dram_tensor("wg0", [D, HH], BFD, kind="ExternalInput"),
        "wg1": nc.dram_tensor("wg1", [D, HH], BFD, kind="ExternalInput"),
        "wv0": nc.dram_tensor("wv0", [D, HH], BFD, kind="ExternalInput"),
        "wv1": nc.dram_tensor("wv1", [D, HH], BFD, kind="ExternalInput"),
        "wo0": nc.dram_tensor("wo0", [HH, D], BFD, kind="ExternalInput"),
        "wo1": nc.dram_tensor("wo1", [HH, D], BFD, kind="ExternalInput"),
        "y0": nc.dram_tensor("y0", [D, CAP0], BFD, kind="ExternalOutput"),
        "y1": nc.dram_tensor("y1", [D, CAP1], BFD, kind="ExternalOutput"),
    }
    with tile.TileContext(nc) as tc:
        with ExitStack() as ctx:
            _emit(nc, tc, ctx, tens)
    _dedup_ldweights(nc)
    nc.compile()
    return nc


_NC = None


def _get_nc():
    global _NC
    if _NC is None:
        _NC = _build()
    return _NC


def _route(xf, gate_w, expert_bias):
    """Host-side replica of the reference router."""
    logits = xf @ gate_w + expert_bias          # [N, E] fp32
    m = logits.max(axis=-1, keepdims=True)
    p = np.exp(logits - m)
    p /= p.sum(axis=-1, keepdims=True)
    # ties -> lower index first, matching jax.lax.top_k
    order = np.argsort(-p, axis=-1, kind="stable")[:, :TOP_K]
    rw = np.take_along_axis(p, order, -1)
    rw = rw / (rw.sum(-1, keepdims=True) + np.float32(1e-8))
    return order, rw


def _slot_inputs(xf, order, rw, ids, cap, w_gate, w_value, w_out,
                 expert, half):
    """Build one (expert, H-half) job's device inputs."""
    ids_p = np.zeros(cap, dtype=np.int64)
    ids_p[: len(ids)] = ids
    xt = np.ascontiguousarray(xf[ids_p].T.astype(BF))
    hsl = slice(half * HH, (half + 1) * HH)
    return {
        "x": xt,
        "wg": np.ascontiguousarray(w_gate[expert][:, hsl].astype(BF)),
        "wv": np.ascontiguousarray(w_value[expert][:, hsl].astype(BF)),
        "wo": np.ascontiguousarray(w_out[expert][hsl, :].astype(BF)),
    }


def kernel(x, gate_w, expert_bias, w_gate, w_value, w_out, _trace=False):
    x = np.asarray(x, dtype=np.float32)
    gate_w = np.asarray(gate_w, dtype=np.float32)
    expert_bias = np.asarray(expert_bias, dtype=np.float32)
    w_gate = np.asarray(w_gate, dtype=np.float32)
    w_value = np.asarray(w_value, dtype=np.float32)
    w_out = np.asarray(w_out, dtype=np.float32)

    xf = np.ascontiguousarray(x.reshape(N_TOK, D))
    order, rw = _route(xf, gate_w, expert_bias)
    idx = [np.flatnonzero((order == e).any(axis=-1)) for e in range(E)]

    # slot 0 <- 4 heaviest experts, slot 1 <- 4 lightest
    by_load = sorted(range(E), key=lambda e: -len(idx[e]))
    slot_exp = (by_load[:4], by_load[4:])
    caps = (CAP0, CAP1)
    nts = (NT0, NT1)
    n_rounds = max(
        max(1, math.ceil(len(idx[e]) / caps[s]))
        for s in range(2) for e in slot_exp[s]
    )

    nc = _get_nc()
    out = np.zeros((N_TOK, D), dtype=np.float32)
    last = None
    for r in range(n_rounds):
        in_maps = []
        round_ids = [[], []]
        for c in range(E):
            m = {}
            for s in range(2):
                e = slot_exp[s][c // 2]
                half = c % 2
                ids = idx[e][r * caps[s]:(r + 1) * caps[s]]
                round_ids[s].append(ids)
                job = _slot_inputs(xf, order, rw, ids, caps[s],
                                   w_gate, w_value, w_out, e, half)
                m[f"x{s}"] = job["x"]
                m[f"wg{s}"] = job["wg"]
                m[f"wv{s}"] = job["wv"]
                m[f"wo{s}"] = job["wo"]
            in_maps.append(m)
        res = run_bass_kernel_spmd(
            nc, in_maps, core_ids=list(range(E)),
            trace=bool(_trace), trace_cores=list(range(E)) if _trace else None,
        )
        last = res
        for c in range(E):
            for s in range(2):
                ids = round_ids[s][c]
                if len(ids):
                    e = slot_exp[s][c // 2]
                    sel = order[ids] == e
                    w_tok = np.where(sel[:, 0], rw[ids, 0], rw[ids, 1])
                    yT = res.results[c][f"y{s}"][:, : len(ids)]
                    out[ids] += w_tok[:, None].astype(np.float32) * \
                        yT.T.astype(np.float32)
    if _trace:
        kernel.last_results = last
    return out.reshape(B, T, D)


# revision 31
# speedup vs baseline: 1.0000x; 1.0000x over previous
"""MoE FFN (8 experts, top-2, SwiGLU) Trainium2 kernel — bf16 half-expert.

Sharding: each expert's hidden dim H=2048 is split into two halves; the
16 (expert, H-half) jobs are packed 2-per-core. Slot 0 holds halves of
the 4 heaviest-loaded experts (per the host router) with token capacity
1092, slot 1 the 4 lightest with capacity 1024 — vs 1152 for plain
expert-parallel SPMD, a ~8% cut in padded matmul rows. The two halves
of an expert produce partial y sums the host adds during scatter.

The router (top-2 + combine weights) runs on host, exactly replicating
the reference. FFN matmuls are bf16 (measured PE rate: 1 cycle per
moving row, weight loads fully hidden), end-to-end rel err ~4.5e-3.
Structure per h-tile: a g-pass then v-pass over 8 K-tiles x 2-3 token
chunks sharing PSUM tags generationally, epilogue computes
silu(g)*v -> bf16 hT tiles consumed by phase B; y is stored bf16 and
scaled/accumulated on host.

Self-contained: shapes/sharding hardcoded for
x[2,2048,1024], 8 experts, d_expert=2048, top-2.
"""

import math
from contextlib import ExitStack

import ml_dtypes
import numpy as np

import concourse.mybir as mybir
import concourse.tile as tile
from concourse import bacc
from concourse.bass_utils import run_bass_kernel_spmd

# ---- problem constants --------------------------------------------------
B, T, D = 2, 2048, 1024
N_TOK = B * T          # 4096 tokens
E = 8                  # experts
H = 2048               # expert hidden dim
HH = H // 2            # per-slot hidden half
TOP_K = 2
P = 128

CAP0 = 1092            # slot-0 token capacity (4 heaviest experts)
CAP1 = 1024            # slot-1 token capacity (4 lightest experts)
ND = D // P            # 8  d-tiles
NHH = HH // P          # 8  h-tiles per slot
NT0 = (CAP0 + P - 1) // P   # 9 (last tile 68 tokens)
NT1 = CAP1 // P             # 8

BFD = mybir.dt.bfloat16
FP = mybir.dt.float32
AF = mybir.ActivationFunctionType
OP = mybir.AluOpType
BF = ml_dtypes.bfloat16

CH0 = [(0, 512), (512, 512), (1024, CAP0 - 1024)]
CH1 = [(0, 512), (512, 512)]

def _emit(nc, tc, ctx, tens):
    const = ctx.enter_context(tc.tile_pool(name="const", bufs=1))
    wsb = ctx.enter_context(tc.tile_pool(name="wsb", bufs=1))
    htp = ctx.enter_context(tc.tile_pool(name="htp", bufs=1))
    act = ctx.enter_context(tc.tile_pool(name="act", bufs=2))
    yst = ctx.enter_context(tc.tile_pool(name="yst", bufs=2))

    caps = (CAP0, CAP1)
    chunks = (CH0, CH1)
    nts = (NT0, NT1)
    x_sb, wg_sb, wv_sb, wo_sb = [], [], [], []
    x_ap, wg_ap, wv_ap, wo_ap = [], [], [], []
    for s in range(2):
        x_sb.append(const.tile([P, ND, caps[s]], BFD, name=f"x{s}"))
        x_ap.append(tens[f"x{s}"].ap().rearrange("(j p) c -> p j c", p=P))
        wg_sb.append(wsb.tile([P, ND, HH], BFD, name=f"wg{s}"))
        wg_ap.append(tens[f"wg{s}"].ap().rearrange("(j p) h -> p j h", p=P))
        wv_sb.append(wsb.tile([P, ND, HH], BFD, name=f"wv{s}"))
        wv_ap.append(tens[f"wv{s}"].ap().rearrange("(j p) h -> p j h", p=P))
        wo_sb.append(wsb.tile([P, NHH, D], BFD, name=f"wo{s}"))
        wo_ap.append(tens[f"wo{s}"].ap().rearrange("(j p) d -> p j d", p=P))

    # head-latency-ordered loads: sync carries the critical path in
    # first-use order (small first chunks -> first matmul asap); the
    # scalar queue configures the v-pass head load in parallel. Bulk
    # loads follow on sync so they cannot steal HBM bandwidth from the
    # head.
    nc.sync.dma_start(out=x_sb[0][:, 0:1, 0:512], in_=x_ap[0][:, 0:1, 0:512])
    nc.sync.dma_start(out=wg_sb[0][:, :, 0:P], in_=wg_ap[0][:, :, 0:P])
    nc.sync.dma_start(out=x_sb[0][:, 0:1, 512:CAP0], in_=x_ap[0][:, 0:1, 512:CAP0])
    nc.sync.dma_start(out=x_sb[0][:, 1:2], in_=x_ap[0][:, 1:2])
    nc.sync.dma_start(out=x_sb[0][:, 2:4], in_=x_ap[0][:, 2:4])
    nc.sync.dma_start(out=x_sb[0][:, 4:6], in_=x_ap[0][:, 4:6])
    nc.sync.dma_start(out=x_sb[0][:, 6:8], in_=x_ap[0][:, 6:8])
    nc.scalar.dma_start(out=wv_sb[0][:, :, 0:P], in_=wv_ap[0][:, :, 0:P])
    HQ = (HH - P) // 2 + P  # 576: split the bulk wg/wv columns in two
    nc.sync.dma_start(out=wg_sb[0][:, :, P:HQ], in_=wg_ap[0][:, :, P:HQ])
    nc.sync.dma_start(out=wv_sb[0][:, :, P:HQ], in_=wv_ap[0][:, :, P:HQ])
    nc.sync.dma_start(out=wg_sb[0][:, :, HQ:HH], in_=wg_ap[0][:, :, HQ:HH])
    nc.sync.dma_start(out=wv_sb[0][:, :, HQ:HH], in_=wv_ap[0][:, :, HQ:HH])
    nc.sync.dma_start(out=x_sb[1][:, 0:4], in_=x_ap[1][:, 0:4])
    nc.sync.dma_start(out=x_sb[1][:, 4:8], in_=x_ap[1][:, 4:8])
    nc.sync.dma_start(out=wg_sb[1][:], in_=wg_ap[1])
    nc.sync.dma_start(out=wv_sb[1][:], in_=wv_ap[1])
    nc.sync.dma_start(out=wo_sb[0][:], in_=wo_ap[0])
    nc.sync.dma_start(out=wo_sb[1][:], in_=wo_ap[1])

    ht = [[htp.tile([P, caps[s]], BFD, name=f"ht{s}_{k}") for k in range(NHH)]
          for s in range(2)]

    # PE p-state warm-up on a zeroed tile: no DMA dependency, so the
    # ramp starts as soon as the vector engine can memset (~1us in).
    warmz = const.tile([P, 2 * P], BFD)
    nc.vector.memset(warmz[:], 0.0)
    with ExitStack() as wctx:
        ps_w = wctx.enter_context(tc.tile_pool(name="psw", bufs=1, space="PSUM"))
        warm = ps_w.tile([P, 2 * P], FP, name="warm", tag="warm")
        for _ in range(24):
            nc.tensor.matmul(warm[:], lhsT=warmz[:, 0:P], rhs=warmz[:],
                             start=True, stop=True)

    # ---- phase A: hT[h, tok] = silu(x@wg)^T * (x@wv)^T ------------------
    # g-pass then v-pass share psum tags (generational cycling): sigmoid
    # and t1 run mid-tile on the g results, freeing banks early.
    with ExitStack() as actx:
        ps_a = actx.enter_context(tc.tile_pool(name="psa", bufs=3, space="PSUM"))
        for s in range(2):
            for hk in range(NHH):
                hs = slice(hk * P, (hk + 1) * P)
                pgs = [ps_a.tile([P, cw], FP, name=f"pg{ci}", tag=f"p{ci}",
                                 bufs=(2 if ci == 2 else 3))
                       for ci, (_, cw) in enumerate(chunks[s])]
                pvs = [ps_a.tile([P, cw], FP, name=f"pv{ci}", tag=f"p{ci}",
                                 bufs=(2 if ci == 2 else 3))
                       for ci, (_, cw) in enumerate(chunks[s])]
                if s == 0 and hk == 0:
                    # interleave g/v per j-tile: halves the x consumption
                    # rate so the first tile tracks the incoming x DMAs
                    for j in range(ND):
                        for w_s, pss in ((wg_sb, pgs), (wv_sb, pvs)):
                            lhsT = w_s[s][:, j, hs]
                            for ci, (cs, cw) in enumerate(chunks[s]):
                                nc.tensor.matmul(
                                    pss[ci][:], lhsT=lhsT,
                                    rhs=x_sb[s][:, j, cs:cs + cw],
                                    start=(j == 0), stop=(j == ND - 1))
                else:
                    for w_s, pss in ((wg_sb, pgs), (wv_sb, pvs)):
                        for j in range(ND):
                            lhsT = w_s[s][:, j, hs]
                            for ci, (cs, cw) in enumerate(chunks[s]):
                                nc.tensor.matmul(
                                    pss[ci][:], lhsT=lhsT,
                                    rhs=x_sb[s][:, j, cs:cs + cw],
                                    start=(j == 0), stop=(j == ND - 1))
                for ci, (cs, cw) in enumerate(chunks[s]):
                    sg = act.tile([P, 512], FP, tag="sg")
                    nc.scalar.activation(sg[:, :cw], pgs[ci][:], AF.Sigmoid)
                    t1 = act.tile([P, 512], FP, tag="t1")
                    nc.vector.tensor_tensor(t1[:, :cw], pgs[ci][:],
                                            sg[:, :cw], op=OP.mult)
                    nc.vector.tensor_tensor(ht[s][hk][:, cs:cs + cw],
                                            t1[:, :cw], pvs[ci][:],
                                            op=OP.mult)

    # ---- phase B: yT[d, tok] = wo^T @ hT --------------------------------
    # d-major output: wo tiles are the stationary operand swept over the
    # exact token count (no padded-token rows for the partial tile); the
    # per-token combine weight is applied by the host during scatter.
    with ExitStack() as bctx:
        ps_y = bctx.enter_context(tc.tile_pool(name="psy", bufs=3, space="PSUM"))
        for s in range(2):
            y_ap = tens[f"y{s}"].ap()
            for dt in range(ND):
                dsl = slice(dt * P, (dt + 1) * P)
                pys = [ps_y.tile([P, cw], FP, name=f"py{ci}", tag=f"py{ci}",
                                 bufs=(2 if ci == 2 else 3))
                       for ci, (_, cw) in enumerate(chunks[s])]
                for hj in range(NHH):
                    lhsT = wo_sb[s][:, hj, dsl]
                    for ci, (cs, cw) in enumerate(chunks[s]):
                        nc.tensor.matmul(
                            pys[ci][:], lhsT=lhsT,
                            rhs=ht[s][hj][:, cs:cs + cw],
                            start=(hj == 0), stop=(hj == NHH - 1))
                ysb = yst.tile([P, caps[s]], BFD, tag="y", name="ysb")
                for ci, (cs, cw) in enumerate(chunks[s]):
                    nc.scalar.activation(ysb[:, cs:cs + cw], pys[ci][:],
                                         AF.Copy)
                # alternate store queues: halves the serialized dma_start
                # sequencing in the drain past the last matmul
                eng = nc.gpsimd if (dt % 2 == 0) else nc.scalar
                eng.dma_start(out=y_ap[dsl, :], in_=ysb[:])


def _dedup_ldweights(nc):
    """Drop InstLdweights that reload the exact weights already resident
    in the PE array (identical AP/mode, no intervening clobber, no sync)."""
    removed = 0
    for blk in nc.main_func.blocks:
        last_key = None
        new = []
        for inst in blk.instructions:
            if isinstance(inst, mybir.InstLdweights):
                si = inst.sync_info
                clean = si is None or (not si.on_wait and not si.on_update)
                key = (
                    repr(inst.ins[0]), str(inst.perf_mode),
                    str(inst.is_transpose), str(inst.tile_position),
                    str(inst.tile_size),
                )
                if clean and key == last_key:
                    removed += 1
                    continue
                last_key = key
            elif isinstance(inst, mybir.InstMatmult):
                if inst.ldweights is None or inst.is_transpose:
                    last_key = None
            new.append(inst)
        blk.instructions[:] = new
    return removed


def _build():
    nc = bacc.Bacc("TRN2", target_bir_lowering=False, debug=False)
    tens = {
        "x0": nc.dram_tensor("x0", [D, CAP0], BFD, kind="ExternalInput"),
        "x1": nc.dram_tensor("x1", [D, CAP1], BFD, kind="ExternalInput"),
        "wg0": nc.dram_tensor("wg0", [D, HH], BFD, kind="ExternalInput"),
        "wg1": nc.dram_tensor("wg1", [D, HH], BFD, kind="ExternalInput"),
        "wv0": nc.dram_tensor("wv0", [D, HH], BFD, kind="ExternalInput"),
        "wv1": nc.dram_tensor("wv1", [D, HH], BFD, kind="ExternalInput"),
        "wo0": nc.dram_tensor("wo0", [HH, D], BFD, kind="ExternalInput"),
        "wo1": nc.dram_tensor("wo1", [HH, D], BFD, kind="ExternalInput"),
        "y0": nc.dram_tensor("y0", [D, CAP0], BFD, kind="ExternalOutput"),
        "y1": nc.dram_tensor("y1", [D, CAP1], BFD, kind="ExternalOutput"),
    }
    with tile.TileContext(nc) as tc:
        with ExitStack() as ctx:
            _emit(nc, tc, ctx, tens)
    _dedup_ldweights(nc)
    nc.compile()
    return nc


_NC = None


def _get_nc():
    global _NC
    if _NC is None:
        _NC = _build()
    return _NC


def _route(xf, gate_w, expert_bias):
    """Host-side replica of the reference router."""
    logits = xf @ gate_w + expert_bias          # [N, E] fp32
    m = logits.max(axis=-1, keepdims=True)
    p = np.exp(logits - m)
    p /= p.sum(axis=-1, keepdims=True)
    # ties -> lower index first, matching jax.lax.top_k
    order = np.argsort(-p, axis=-1, kind="stable")[:, :TOP_K]
    rw = np.take_along_axis(p, order, -1)
    rw = rw / (rw.sum(-1, keepdims=True) + np.float32(1e-8))
    return order, rw


def _slot_inputs(xf, order, rw, ids, cap, w_gate, w_value, w_out,
                 expert, half):
    """Build one (expert, H-half) job's device inputs."""
    ids_p = np.zeros(cap, dtype=np.int64)
    ids_p[: len(ids)] = ids
    xt = np.ascontiguousarray(xf[ids_p].T.astype(BF))
    hsl = slice(half * HH, (half + 1) * HH)
    return {
        "x": xt,
        "wg": np.ascontiguousarray(w_gate[expert][:, hsl].astype(BF)),
        "wv": np.ascontiguousarray(w_value[expert][:, hsl].astype(BF)),
        "wo": np.ascontiguousarray(w_out[expert][hsl, :].astype(BF)),
    }


def kernel(x, gate_w, expert_bias, w_gate, w_value, w_out, _trace=False):
    x = np.asarray(x, dtype=np.float32)
    gate_w = np.asarray(gate_w, dtype=np.float32)
    expert_bias = np.asarray(expert_bias, dtype=np.float32)
    w_gate = np.asarray(w_gate, dtype=np.float32)
    w_value = np.asarray(w_value, dtype=np.float32)
    w_out = np.asarray(w_out, dtype=np.float32)

    xf = np.ascontiguousarray(x.reshape(N_TOK, D))
    order, rw = _route(xf, gate_w, expert_bias)
    idx = [np.flatnonzero((order == e).any(axis=-1)) for e in range(E)]

    # slot 0 <- 4 heaviest experts, slot 1 <- 4 lightest
    by_load = sorted(range(E), key=lambda e: -len(idx[e]))
    slot_exp = (by_load[:4], by_load[4:])
    caps = (CAP0, CAP1)
    nts = (NT0, NT1)
    n_rounds = max(
        max(1, math.ceil(len(idx[e]) / caps[s]))
        for s in range(2) for e in slot_exp[s]
    )

    nc = _get_nc()
    out = np.zeros((N_TOK, D), dtype=np.float32)
    last = None
    for r in range(n_rounds):
        in_maps = []
        round_ids = [[], []]
        for c in range(E):
            m = {}
            for s in range(2):
                e = slot_exp[s][c // 2]
                half = c % 2
                ids = idx[e][r * caps[s]:(r + 1) * caps[s]]
                round_ids[s].append(ids)
                job = _slot_inputs(xf, order, rw, ids, caps[s],
                                   w_gate, w_value, w_out, e, half)
                m[f"x{s}"] = job["x"]
                m[f"wg{s}"] = job["wg"]
                m[f"wv{s}"] = job["wv"]
                m[f"wo{s}"] = job["wo"]
            in_maps.append(m)
        res = run_bass_kernel_spmd(
            nc, in_maps, core_ids=list(range(E)),
            trace=bool(_trace), trace_cores=list(range(E)) if _trace else None,
        )
        last = res
        for c in range(E):
            for s in range(2):
                ids = round_ids[s][c]
                if len(ids):
                    e = slot_exp[s][c // 2]
                    sel = order[ids] == e
                    w_tok = np.where(sel[:, 0], rw[ids, 0], rw[ids, 1])
                    yT = res.results[c][f"y{s}"][:, : len(ids)]
                    out[ids] += w_tok[:, None].astype(np.float32) * \
                        yT.T.astype(np.float32)
    if _trace:
        kernel.last_results = last
    return out.reshape(B, T, D)
